# revision 74
# baseline (speedup 1.0000x reference)
"""Trainium2 Bass kernel for nn_ARSG (additive-attention style scoring with a
1-D conv over location features), data-parallel over batch across 8 NeuronCores.

Math (per batch b):
    f      = conv1d(F_matrix, a_prev[b])          # Toeplitz matmul over T
    x      = tanh(s_prev[b] @ Ww + hT[b] @ Vw + Vb + f @ Uw)
    e      = x @ ww
    out[b] = softmax(beta * e)

Key restructurings (validated vs the reference in fp64/fp32 mock):
  * Uw is folded into F on the host: G = F @ Uw, so Uf^T = G^T @ C_b^T where
    C_b^T is the (banded Toeplitz) conv coefficient matrix built from a_prev.
    This removes the separate f @ Uw matmul entirely (-25% FLOPs).
  * C_b^T tiles are materialized by DMA directly from a reversed, zero-padded
    copy of a_prev ("qrev") using an overlapping [1,128]x[1,512] access
    pattern.  Both matmul operands have their K-partitions reversed per
    128-block (G is block-reversed on the host), which keeps all AP steps
    positive while leaving the contraction sum unchanged.
  * h is transposed on the host to [b, DIM_H, T] so Vh^T accumulates into the
    same PSUM tile as Uf^T with K = DIM_H on partitions.
  * s_prev @ Ww + Vb (tiny) is computed on the host and applied as the
    per-partition bias of the tanh activation.
  * The conv pair (G, qrev) runs in fp8e4 with perf_mode=DoubleRow: each
    matmul contracts TWO 128-K-blocks at 2 rows/cycle (~2x bf16 ALU rate).
    The conv coefficients are scaled by SC=1024 on the host (softmax probs
    ~1e-3 would otherwise land in fp8 subnormals); Vw is pre-scaled by SC
    too so the shared PSUM is uniformly SC-scaled, and the tanh activation
    applies scale=1/SC (bias is added after the scale, so it stays unscaled).
    The Vh and e matmuls run as float32r (full fp32 data; reduced-precision
    PE mode, 1 cycle/row at N>=256 -- same rate as bf16, ~4x faster than
    fp32).  fp8 for Vh was host-simulated at rel err 0.041 > 2e-2 tolerance,
    so Vh stays f32r.
  * The final softmax divide runs on the HOST: the device ships the
    unnormalized exp(beta*e) rows; the host sums+divides (exactly the
    reference's softmax tail).  This trims the device tail (reciprocal +
    multiply + extra DMA hop) off the critical path.
  * Inputs are packed partition-major on the host so each tensor loads with
    1-2 large DMAs (2-8 KiB per partition line) instead of many small ones:
    the Sync sequencer's ~600ns-per-DMA issue cost dominated startup.
    vw + the bias/ww constants issue from the Scalar sequencer (also HWDGE)
    in parallel with the Sync-issued conv-critical loads.
  * Startup warmup: dummy matmuls keep the PE busy from ~1us until the first
    real conv matmul's data lands, so the HAM activity window un-throttles
    the PE clock (1.2 -> 2.4 GHz) right as the real stream begins.  The
    baseline had a 750ns idle gap here which delayed un-throttle by ~10us.

Everything below T/B/... is hardcoded for the problem sizes:
    T=1024, B=32, DIM_F=512, DIM_H=512, DIM_S=1024, DIM_W=512, 8 cores.
"""

import numpy as np

T, B, DIM_F, DIM_H, DIM_S, DIM_W = 1024, 32, 512, 512, 1024, 512
N_CORES = 8
B_LOC = B // N_CORES  # batches per core
QLEN = 2048           # padded length of the reversed conv-coefficient vector
SC = 1024.0           # fp8 conv coefficient scale (power of 2, undone in tanh)
N_WARM = 12           # startup dummy matmuls (N=512) bridging from ~1.6us to
                      # the first real matmul (~6.9us); cold dummies run 427ns

_program_cache: dict[float, object] = {}


def _build_program(beta: float):
    import concourse.bass as bass
    import concourse.mybir as mybir
    import concourse.tile as tile
    from concourse import bacc

    f32 = mybir.dt.float32
    f32r = mybir.dt.float32r
    bf16 = mybir.dt.bfloat16
    fp8 = mybir.dt.float8e4
    DR = mybir.MatmulPerfMode.DoubleRow
    AFT = mybir.ActivationFunctionType

    nc = bacc.Bacc("TRN2", target_bir_lowering=False, debug=False)

    NKJ = T // 128       # 8 K-blocks for the conv contraction (over j)
    NKD = DIM_H // 128   # 4 K-blocks for the Vh contraction (over d)
    NWT = DIM_W // 128   # 4 output w-tiles
    NTC = T // 512       # 2 t-chunks of 512 (PSUM bank / fp32 moving-max)

    # Partition-major packed inputs: one/two big DMAs per tensor.
    g_d = nc.dram_tensor("g", [128, NKJ * DIM_W], fp8, kind="ExternalInput")
    vw_d = nc.dram_tensor("vw", [128, NKD * DIM_W], bf16,
                          kind="ExternalInput")
    # h ships as bf16 (halves the dominant DMA stream) and feeds the Vh
    # matmuls directly: bf16 matmuls run at the same 1 row/cycle as f32r,
    # and host-sim puts bf16-Vh accuracy at rel err 0.0025 (same as the
    # fp8-conv floor), so the f32 upcast pass the baseline ran on the
    # Vector engine is pure overhead.
    ht_d = nc.dram_tensor("ht", [B_LOC, 128 * NKD * T], bf16,
                          kind="ExternalInput")
    qr_d = nc.dram_tensor("qrev", [B_LOC, QLEN], fp8, kind="ExternalInput")
    bias_d = nc.dram_tensor("bias", [128, B_LOC * 4], f32,
                            kind="ExternalInput")
    # col 4 is all-ones: the lhsT of the partition-sum matmul in the e path.
    wwr_d = nc.dram_tensor("wwr", [128, 5], f32r, kind="ExternalInput")
    # unnormalized exp(beta*e); host does the softmax sum+divide.
    out_d = nc.dram_tensor("out", [B_LOC, T], f32, kind="ExternalOutput")

    with tile.TileContext(nc) as tc:
        with (
            tc.tile_pool(name="const", bufs=1) as const_pool,
            tc.tile_pool(name="htbp", bufs=2) as htb_pool,
            tc.tile_pool(name="convp", bufs=3) as conv_pool,
            tc.tile_pool(name="xp", bufs=4) as x_pool,
            tc.tile_pool(name="ep", bufs=2) as e_pool,
            tc.tile_pool(name="sp", bufs=3) as s_pool,
            tc.tile_pool(name="smallp", bufs=4) as small_pool,
            tc.tile_pool(name="psx", bufs=7, space="PSUM") as psx_pool,
            tc.tile_pool(name="pse", bufs=1, space="PSUM") as pse_pool,
        ):
            # All conv coefficient tiles for a batch are overlapping windows
            # of qrev inside ONE [128, 1920] window: W[p, c] = qrev[lb, p+c];
            # the rhs for (kj, tch) is W[:, 512*tch + 128*(NKJ-1-kj) :+ 512].
            def load_w_piece(w_sb, lb, c0, c1, eng=None):
                (eng or nc.sync).dma_start(
                    out=w_sb[:, c0:c1],
                    in_=bass.AP(tensor=qr_d, offset=lb * QLEN + c0,
                                ap=[[1, 128], [1, c1 - c0]]),
                )

            def load_w(lb):
                w_sb = conv_pool.tile([128, 1920], fp8, tag="conv",
                                      name=f"w_{lb}")
                load_w_piece(w_sb, lb, 0, 1920)
                return w_sb

            g_sb = const_pool.tile([128, NKJ, DIM_W], fp8)

            def load_g(s0, s1, eng=None):  # slot range [s0, s1): one DMA
                # tile-slice out AP so reads of other slots don't pick up a
                # false dependency on this DMA
                (eng or nc.sync).dma_start(
                    out=g_sb[:, s0:s1, :],
                    in_=bass.AP(tensor=g_d, offset=s0 * DIM_W,
                                ap=[[NKJ * DIM_W, 128], [1, (s1 - s0) * DIM_W]]),
                )

            # h loads: one packed DMA per batch (8 KiB per partition line);
            # the bf16 tile feeds the Vh matmuls directly.  Batch 0 splits
            # per-kd so the early-kd Vh matmuls can start (keeping the PE
            # busy) while the later slices are still in flight.
            def load_ht(lb, split_dma=False):
                htb_sb = htb_pool.tile([128, NKD * T], bf16, tag="htb",
                                       name=f"htb_{lb}")
                if split_dma:
                    for kd in range(NKD):
                        nc.sync.dma_start(
                            out=htb_sb[:, kd * T:(kd + 1) * T],
                            in_=bass.AP(tensor=ht_d,
                                        offset=lb * 128 * NKD * T + kd * T,
                                        ap=[[NKD * T, 128], [1, T]]),
                        )
                else:
                    nc.sync.dma_start(
                        out=htb_sb[:],
                        in_=bass.AP(tensor=ht_d, offset=lb * 128 * NKD * T,
                                    ap=[[NKD * T, 128], [1, NKD * T]]),
                    )
                return htb_sb

            vw_sb = const_pool.tile([128, NKD, DIM_W], bf16)
            bias_sb = const_pool.tile([128, B_LOC * 4], f32)
            wwr_sb = const_pool.tile([128, 5], f32r)

            # HAM warmup: the PE idles ~4us waiting for the first loads, and
            # whatever runs in the first ~3.4us of PE activity runs at the
            # cold 1.2GHz clock.  Spend that window on dummy matmuls over a
            # zeroed scratch tile so the real matmuls start at 2.4GHz; the
            # dummies and the real stream must stay gap-free (a fully-idle
            # 3.4us HAM window re-throttles).
            warm_in = const_pool.tile([128, 640], bf16)
            nc.vector.memset(warm_in[:], 0.0)

            # Startup DMA issue splits across BOTH HWDGE sequencers so the
            # two first-needed transfers (g pairs 3,2 and the w0 window)
            # land together ~6us in: Sync takes g + ht0, Scalar takes w0 +
            # vw + constants.  Per-DMA issue-to-complete is ~5.3us here, so
            # issue order IS the completion order.
            # tch0's conv windows span w cols [0, 1408): pair p at tch reads
            # cols [512*tch + 128*(6-2p), +640).
            # CAUTION: this exact issue order matters beyond overlap — runs
            # that front-load the 1MB ht0 stream into the first issue slots
            # (before g-lo) latch the WHOLE run into the P0 power state
            # (PE at ~2.0GHz instead of 2.4 -> +16us).  Measured repeatedly;
            # keep ht0 after g-lo even though its late slices cost ~2.5us.
            # (g-hi via GpSimd/SWDGE was tried: the SWDGE transfer lands
            # LATER than HWDGE's, net +1.6us.  All loads stay HWDGE.)
            load_g(4, NKJ)                 # pairs 3,2 (first conv matmuls)
            w0_sb = conv_pool.tile([128, 1920], fp8, tag="conv", name="w_0")
            load_w_piece(w0_sb, 0, 0, 1408, eng=nc.scalar)  # tch0 windows
            load_g(0, 4)                        # pairs 1,0
            ht0_sb = load_ht(0, split_dma=True)
            load_w_piece(w0_sb, 0, 1408, 1920)  # rest (tch1 windows)

            warm_ps = psx_pool.tile([128, 512], f32, tag="psx", name="warm_ps")
            for _ in range(N_WARM):
                nc.tensor.matmul(warm_ps[:], warm_in[:, 0:128],
                                 warm_in[:, 128:640], start=True, stop=True)
            # Dummy activations so the Tanh/Exp act-table loads happen during
            # the startup DMA wait instead of right before the first use.
            aw_sb = small_pool.tile([1, 2], f32, tag="aw", name="aw")
            nc.scalar.activation(aw_sb[:, 0:1], warm_in[0:1, 0:1], AFT.Tanh)
            nc.scalar.activation(aw_sb[:, 1:2], warm_in[0:1, 0:1], AFT.Exp)

            nc.scalar.dma_start(
                out=vw_sb[:],
                in_=bass.AP(tensor=vw_d, offset=0,
                            ap=[[NKD * DIM_W, 128], [1, NKD * DIM_W]]),
            )
            nc.scalar.dma_start(out=bias_sb[:], in_=bias_d.ap()[:])
            nc.scalar.dma_start(out=wwr_sb[:], in_=wwr_d.ap()[:])

            # --- per-batch emission helpers -------------------------------
            # Conv runs in fp8e4 DoubleRow: each matmul contracts a PAIR of
            # 128-K-blocks at 2 rows/cycle.  g_sb slot order is
            # host-arranged [kj1,kj0,kj3,kj2,...] so pair p's dim1 is
            # (kj_hi=2p+1, kj_lo=2p) matching the rhs windows at
            # (c0, c0+128).  Pairs p>=2 first: at tch==0 their t=0
            # coefficients are naturally zero, so the start=True matmul
            # covers the full 512 columns; pairs p<2 (kj<4) carry junk t=0
            # coefficients (the t=0 conv output row is zero by construction
            # -- even-T padding in the reference), so they skip column 0
            # (N=511, odd rhs offset) and just accumulate.
            # Pair-major emission (all wt groups' pair p before pair p-1):
            # if a load for a later pair is still in flight, every ready
            # matmul ahead of it still issues (the PE queue is in-order, so
            # group-major order would head-of-line-block on the g/w DMAs
            # during batch 0).
            def emit_conv_pair(w_sb, pss, tch, p, wts, start):
                c0 = 512 * tch + 128 * (NKJ - 2 - 2 * p)
                skip = 1 if (tch == 0 and p < 2) else 0
                rhs = bass.AP(
                    tensor=w_sb.tensor,
                    offset=w_sb.offset + c0 + skip,
                    ap=[list(w_sb.ap[0]), [128, 2], [1, 512 - skip]],
                )
                for wt in wts:
                    nc.tensor.matmul(
                        pss[wt][:, skip:],
                        g_sb[:, 2 * p:2 * p + 2, wt * 128:(wt + 1) * 128],
                        rhs, start=start, stop=False, perf_mode=DR,
                    )

            def emit_conv(w_sb, pss, tch, pairs=(3, 2, 1, 0)):
                for p in pairs:
                    emit_conv_pair(w_sb, pss, tch, p, range(NWT), p == 3)

            def emit_vh(htb_sb, ps, tch, wt):
                for kd in range(NKD):
                    c = kd * T + tch * 512
                    nc.tensor.matmul(
                        ps[:],
                        vw_sb[:, kd, wt * 128:(wt + 1) * 128],
                        htb_sb[:, c:c + 512],
                        start=False, stop=(kd == NKD - 1),
                    )

            def emit_act(lb, ps, x_sb, wt):
                nc.scalar.activation(
                    x_sb[:, wt, :], ps[:], AFT.Tanh,
                    bias=bias_sb[:, lb * 4 + wt: lb * 4 + wt + 1],
                    scale=1.0 / SC,
                )

            def emit_e_offload(lb, tch, pe_t, x_sb):
                # Reduce over wt on the Vector engine (s = sum_wt
                # x_wt * ww_wt), then one K=128 ones-matmul does the
                # partition sum: frees ~1.4us/batch of PE time.
                # (gpsimd.partition_all_reduce was measured at 3.5us per
                # call -- 16x the ones-matmul -- so the PE hop stays.  A
                # bf16 ones-matmul variant measured net-slower: the 400ns
                # block-boundary gaps are dependency waits, not dtype-mode
                # switches, and the bf16 s hop added two ~309ns waits.)
                s_sb = s_pool.tile([128, 512], f32r, tag="s",
                                   name=f"s_{lb}_{tch}")
                nc.vector.tensor_scalar_mul(
                    s_sb[:], x_sb[:, 0, :].bitcast(f32),
                    wwr_sb[:, 0:1].bitcast(f32))
                for wt in range(1, NWT):
                    nc.vector.scalar_tensor_tensor(
                        s_sb[:], x_sb[:, wt, :].bitcast(f32),
                        wwr_sb[:, wt:wt + 1].bitcast(f32),
                        s_sb[:].bitcast(f32),
                        op0=mybir.AluOpType.mult,
                        op1=mybir.AluOpType.add)
                nc.tensor.matmul(pe_t[:], wwr_sb[:, 4:5], s_sb[:],
                                 start=True, stop=True)
                return pe_t[:]

            def emit_e_pe(pe_t, x_sb):
                for wt in range(NWT):
                    nc.tensor.matmul(
                        pe_t[:], wwr_sb[:, wt:wt + 1], x_sb[:, wt, :],
                        start=(wt == 0), stop=(wt == NWT - 1),
                    )
                return pe_t[:]

            # beta is small (|e| <~ 20): exp(beta*e) cannot overflow, so no
            # max-subtraction pass; each t-half is exponentiated and shipped
            # right after its e matmul.  Host sums + divides.
            assert abs(beta) <= 4.0, "large-beta path removed (host softmax)"

            def emit_exp_out(lb, tch, e_src, p_sb, raw=False):
                if raw:
                    # tail-exposed slice: ship RAW e via a ~200ns DVE copy
                    # (PSUM->SBUF) instead of the 678ns Scalar exp; the host
                    # applies exp to this one slice (see _RAW_SLICE).
                    nc.vector.tensor_copy(
                        out=p_sb[:, tch * 512:(tch + 1) * 512], in_=e_src)
                else:
                    nc.scalar.activation(
                        p_sb[:, tch * 512:(tch + 1) * 512],
                        e_src, AFT.Exp, scale=float(beta),
                    )
                nc.sync.dma_start(
                    out=out_d.ap()[lb:lb + 1, tch * 512:(tch + 1) * 512],
                    in_=p_sb[:, tch * 512:(tch + 1) * 512])

            # Each block's EXP is emitted one block LATE: the Scalar engine
            # is FIFO, and an exp placed between two blocks' tanh groups
            # stalls the later tanhs on the e-reduction chain (DVE s ops ->
            # ones-matmul) it waits for; that delays the PSUM-bank frees the
            # next conv group needs.  Deferring the exp one block gives its
            # inputs a full block of slack.  The last batch flushes inline
            # (shortest exposed tail).
            pending_exp = None
            fill1 = None  # batch-0 tch1 PSUM tiles pre-started during tch0
            for lb in range(B_LOC):
                htb_sb = ht0_sb if lb == 0 else load_ht(lb)
                w_sb = w0_sb if lb == 0 else load_w(lb)
                p_sb = e_pool.tile([1, T], f32, tag="p", name=f"p_{lb}")
                offload = lb < B_LOC - 1
                for tch in range(NTC):
                    x_sb = x_pool.tile([128, NWT, 512], f32r, tag="x",
                                       name=f"x_{lb}_{tch}")
                    if fill1 is not None and tch == 1:
                        pss = fill1
                        fill1 = None
                        # finish what the tch0-time fill couldn't start:
                        # wt3 (its PSUM bank frees with tanh(tch0,wt0)) and
                        # pairs 1,0 (which need the w0B window).
                        emit_conv_pair(w_sb, pss, 1, 3, [3], True)
                        emit_conv_pair(w_sb, pss, 1, 2, [3], False)
                        emit_conv(w_sb, pss, 1, pairs=(1, 0))
                    else:
                        pss = []
                        for wt in range(NWT):
                            ps = psx_pool.tile([128, 512], f32, tag="psx",
                                               name=f"ps_{lb}_{tch}_{wt}")
                            pss.append(ps)
                        # NOTE: interleaving pair 0 with the Vh groups (to
                        # advance each group's stop) measured net-SLOWER
                        # even at full clock (77.1us, stream excess 8.1us
                        # vs 3.6us) — the per-group pair-0 emission creates
                        # more wait exposure than it removes.  Keep the
                        # plain pair-major order.
                        emit_conv(w_sb, pss, tch)
                    if lb == 0 and tch == 0:
                        # Fill the ht0-landing wait with tch1 conv work that
                        # needs only the already-loaded g + w0A data: pairs
                        # 3,2 for wt 0..2 (wt3's bank isn't free yet).
                        fill1 = []
                        for wt in range(NWT):
                            ps = psx_pool.tile([128, 512], f32, tag="psx",
                                               name=f"ps_0_1_{wt}")
                            fill1.append(ps)
                        emit_conv_pair(w_sb, fill1, 1, 3, [0, 1, 2], True)
                        emit_conv_pair(w_sb, fill1, 1, 2, [0, 1, 2], False)
                    for wt in range(NWT):
                        emit_vh(htb_sb, pss[wt], tch, wt)
                        emit_act(lb, pss[wt], x_sb, wt)
                    if pending_exp is not None:
                        emit_exp_out(*pending_exp)
                        pending_exp = None
                    # the last batch's tch1 e stays on the PE (the DVE hop
                    # would lengthen the exposed tail); its tch0 e is not
                    # tail-exposed, so it offloads like the others
                    pe_t = pse_pool.tile([1, 512], f32, tag="pse",
                                         name=f"pe_{lb}_{tch}")
                    if offload or tch == 0:
                        e_src = emit_e_offload(lb, tch, pe_t, x_sb)
                    else:
                        e_src = emit_e_pe(pe_t, x_sb)
                    if lb == B_LOC - 1 and tch == 1:
                        # raw-e shipping (DVE copy + host exp) for this
                        # slice was tried and measured neutral-to-worse
                        # (76.3-76.7 vs 74.9-75.9); the tail is quantized
                        # by the end-barrier + epilogue, so keep the exp.
                        # The last batch's tch0 exp defers like the others:
                        # emitted inline it sits in the Scalar FIFO ahead
                        # of tch1's tanhs, stalling them on its e-chain.
                        emit_exp_out(lb, tch, e_src, p_sb)
                    else:
                        pending_exp = (lb, tch, e_src, p_sb)

    nc.compile()
    return nc


def _get_program(beta: float):
    if beta not in _program_cache:
        _program_cache[beta] = _build_program(beta)
    return _program_cache[beta]


def _prepare_in_maps(F, a_prev, s_prev, h, Ww, Vw, Vb, Uw, ww):
    """Host-side sharding + layout prep. Cheap (one small matmul + copies)."""
    import ml_dtypes
    e4 = ml_dtypes.float8_e4m3
    bf16 = ml_dtypes.bfloat16
    G = (F.astype(np.float64) @ Uw.astype(np.float64)).astype(np.float32)
    # Reverse each 128-row block of G so conv lhsT/rhs partition orders match,
    # then swap each even/odd kj block so DoubleRow pair p's lhsT dim1 order
    # is (kj_hi=2p+1, kj_lo=2p), matching rhs windows at (c0, c0+128).
    G_br = G.reshape(T // 128, 128, DIM_W)[:, ::-1, :]
    G_br = G_br[[1, 0, 3, 2, 5, 4, 7, 6]]
    # partition-major packing: g_pk[p, slot, :] = G_br[slot*128 + p, :]
    g_pk = np.ascontiguousarray(
        G_br.astype(e4).transpose(1, 0, 2).reshape(128, T // 128 * DIM_W))
    Ws = (s_prev.astype(np.float64) @ Ww.astype(np.float64)).astype(np.float32)
    Ws = Ws + Vb[None, :]                                   # [B, DIM_W]
    # Vh accumulates into the same PSUM as the SC-scaled fp8 conv, so Vw is
    # pre-scaled by SC (exact, power of 2); the tanh activation undoes it.
    # partition-major packing: vw_pk[p, kd, :] = Vw[kd*128 + p, :] * SC
    vw_pk = np.ascontiguousarray(
        (Vw * np.float32(SC)).astype(bf16).reshape(4, 128, DIM_W)
        .transpose(1, 0, 2).reshape(128, 4 * DIM_W))

    in_maps = []
    for core in range(N_CORES):
        b0 = core * B_LOC
        ppad = np.zeros((B_LOC, 2 * T - 1), np.float32)
        ppad[:, T // 2 - 1: T // 2 - 1 + T] = a_prev[b0:b0 + B_LOC]
        qrev = np.zeros((B_LOC, QLEN), np.float32)
        qrev[:, : 2 * T - 1] = ppad[:, ::-1]
        qrev = (qrev * np.float32(SC)).astype(e4)
        # ht_pk[lb, p, kd, t] = h[t, b0+lb, kd*128+p]  (partition-major)
        hT = (h[:, b0:b0 + B_LOC, :].transpose(1, 2, 0).astype(bf16)
              .reshape(B_LOC, 4, 128, T).transpose(0, 2, 1, 3)
              .reshape(B_LOC, 128 * 4 * T))
        hT = np.ascontiguousarray(hT)
        bias_core = np.ascontiguousarray(
            Ws[b0:b0 + B_LOC].reshape(B_LOC, 4, 128)
            .transpose(2, 0, 1).reshape(128, B_LOC * 4))
        # [128, 5]: cols 0-3 = ww blocks, col 4 = ones (e partition-sum lhsT)
        wwr = np.ascontiguousarray(np.concatenate(
            [ww.reshape(4, 128).T, np.ones((128, 1), np.float32)], axis=1))
        in_maps.append({
            "g": g_pk, "vw": vw_pk, "qrev": qrev, "ht": hT,
            "bias": bias_core, "wwr": wwr,
        })
    return in_maps


def _finish(p_cores: list, beta: float) -> np.ndarray:
    """p_cores: per-core [B_LOC, T] unnormalized exp(beta*e) rows from the
    device. Host does the softmax sum+divide (the reference's exact tail)."""
    p = np.concatenate(p_cores, axis=0).astype(np.float64)
    return (p / p.sum(axis=1, keepdims=True)).astype(np.float32)


def kernel(**inputs: np.ndarray) -> np.ndarray:
    F = np.ascontiguousarray(np.asarray(inputs["F_matrix"], dtype=np.float32))
    a_prev = np.ascontiguousarray(np.asarray(inputs["a_prev"], dtype=np.float32))
    s_prev = np.ascontiguousarray(np.asarray(inputs["s_prev"], dtype=np.float32))
    h = np.ascontiguousarray(np.asarray(inputs["h"], dtype=np.float32))
    Ww = np.asarray(inputs["Ww"], dtype=np.float32)
    Vw = np.asarray(inputs["Vw"], dtype=np.float32)
    Vb = np.asarray(inputs["Vb"], dtype=np.float32)
    Uw = np.asarray(inputs["Uw"], dtype=np.float32)
    ww = np.asarray(inputs["ww"], dtype=np.float32)
    beta = float(np.asarray(inputs["beta"]))

    nc = _get_program(beta)
    in_maps = _prepare_in_maps(F, a_prev, s_prev, h, Ww, Vw, Vb, Uw, ww)

    from concourse.bass_utils import run_bass_kernel_spmd

    res = run_bass_kernel_spmd(nc, in_maps, core_ids=list(range(N_CORES)))
    return _finish([res.results[i]["out"] for i in range(N_CORES)], beta)


# revision 75
# speedup vs baseline: 1.0050x; 1.0050x over previous
"""Trainium2 Bass kernel for nn_ARSG (additive-attention style scoring with a
1-D conv over location features), data-parallel over batch across 8 NeuronCores.

Math (per batch b):
    f      = conv1d(F_matrix, a_prev[b])          # Toeplitz matmul over T
    x      = tanh(s_prev[b] @ Ww + hT[b] @ Vw + Vb + f @ Uw)
    e      = x @ ww
    out[b] = softmax(beta * e)

Key restructurings (validated vs the reference in fp64/fp32 mock):
  * Uw is folded into F on the host: G = F @ Uw, so Uf^T = G^T @ C_b^T where
    C_b^T is the (banded Toeplitz) conv coefficient matrix built from a_prev.
    This removes the separate f @ Uw matmul entirely (-25% FLOPs).
  * C_b^T tiles are materialized by DMA directly from a reversed, zero-padded
    copy of a_prev ("qrev") using an overlapping [1,128]x[1,512] access
    pattern.  Both matmul operands have their K-partitions reversed per
    128-block (G is block-reversed on the host), which keeps all AP steps
    positive while leaving the contraction sum unchanged.
  * h is transposed on the host to [b, DIM_H, T] so Vh^T accumulates into the
    same PSUM tile as Uf^T with K = DIM_H on partitions.
  * s_prev @ Ww + Vb (tiny) is computed on the host and applied as the
    per-partition bias of the tanh activation.
  * The conv pair (G, qrev) runs in fp8e4 with perf_mode=DoubleRow: each
    matmul contracts TWO 128-K-blocks at 2 rows/cycle (~2x bf16 ALU rate).
    The conv coefficients are scaled by SC=1024 on the host (softmax probs
    ~1e-3 would otherwise land in fp8 subnormals); Vw is pre-scaled by SC
    too so the shared PSUM is uniformly SC-scaled, and the tanh activation
    applies scale=1/SC (bias is added after the scale, so it stays unscaled).
    The Vh and e matmuls run as float32r (full fp32 data; reduced-precision
    PE mode, 1 cycle/row at N>=256 -- same rate as bf16, ~4x faster than
    fp32).  fp8 for Vh was host-simulated at rel err 0.041 > 2e-2 tolerance,
    so Vh stays f32r.
  * The final softmax divide runs on the HOST: the device ships the
    unnormalized exp(beta*e) rows; the host sums+divides (exactly the
    reference's softmax tail).  This trims the device tail (reciprocal +
    multiply + extra DMA hop) off the critical path.
  * Inputs are packed partition-major on the host so each tensor loads with
    1-2 large DMAs (2-8 KiB per partition line) instead of many small ones:
    the Sync sequencer's ~600ns-per-DMA issue cost dominated startup.
    vw + the bias/ww constants issue from the Scalar sequencer (also HWDGE)
    in parallel with the Sync-issued conv-critical loads.
  * Startup warmup: dummy matmuls keep the PE busy from ~1us until the first
    real conv matmul's data lands, so the HAM activity window un-throttles
    the PE clock (1.2 -> 2.4 GHz) right as the real stream begins.  The
    baseline had a 750ns idle gap here which delayed un-throttle by ~10us.

Everything below T/B/... is hardcoded for the problem sizes:
    T=1024, B=32, DIM_F=512, DIM_H=512, DIM_S=1024, DIM_W=512, 8 cores.
"""

import numpy as np

T, B, DIM_F, DIM_H, DIM_S, DIM_W = 1024, 32, 512, 512, 1024, 512
N_CORES = 8
B_LOC = B // N_CORES  # batches per core
QLEN = 2048           # padded length of the reversed conv-coefficient vector
SC = 1024.0           # fp8 conv coefficient scale (power of 2, undone in tanh)
N_WARM = 12           # startup dummy matmuls (N=512) bridging from ~1.6us to
                      # the first real matmul (~6.9us); cold dummies run 427ns

_program_cache: dict[float, object] = {}


def _build_program(beta: float):
    import concourse.bass as bass
    import concourse.mybir as mybir
    import concourse.tile as tile
    from concourse import bacc

    f32 = mybir.dt.float32
    f32r = mybir.dt.float32r
    bf16 = mybir.dt.bfloat16
    fp8 = mybir.dt.float8e4
    DR = mybir.MatmulPerfMode.DoubleRow
    AFT = mybir.ActivationFunctionType

    nc = bacc.Bacc("TRN2", target_bir_lowering=False, debug=False)

    NKJ = T // 128       # 8 K-blocks for the conv contraction (over j)
    NKD = DIM_H // 128   # 4 K-blocks for the Vh contraction (over d)
    NWT = DIM_W // 128   # 4 output w-tiles
    NTC = T // 512       # 2 t-chunks of 512 (PSUM bank / fp32 moving-max)

    # Partition-major packed inputs: one/two big DMAs per tensor.
    g_d = nc.dram_tensor("g", [128, NKJ * DIM_W], fp8, kind="ExternalInput")
    vw_d = nc.dram_tensor("vw", [128, NKD * DIM_W], bf16,
                          kind="ExternalInput")
    # h ships as bf16 (halves the dominant DMA stream) and feeds the Vh
    # matmuls directly: bf16 matmuls run at the same 1 row/cycle as f32r,
    # and host-sim puts bf16-Vh accuracy at rel err 0.0025 (same as the
    # fp8-conv floor), so the f32 upcast pass the baseline ran on the
    # Vector engine is pure overhead.
    ht_d = nc.dram_tensor("ht", [B_LOC, 128 * NKD * T], bf16,
                          kind="ExternalInput")
    qr_d = nc.dram_tensor("qrev", [B_LOC, QLEN], fp8, kind="ExternalInput")
    bias_d = nc.dram_tensor("bias", [128, B_LOC * 4], f32,
                            kind="ExternalInput")
    # col 4 is all-ones: the lhsT of the partition-sum matmul in the e path.
    wwr_d = nc.dram_tensor("wwr", [128, 5], f32r, kind="ExternalInput")
    # unnormalized exp(beta*e); host does the softmax sum+divide.
    out_d = nc.dram_tensor("out", [B_LOC, T], f32, kind="ExternalOutput")

    with tile.TileContext(nc) as tc:
        with (
            tc.tile_pool(name="const", bufs=1) as const_pool,
            tc.tile_pool(name="htbp", bufs=2) as htb_pool,
            tc.tile_pool(name="convp", bufs=3) as conv_pool,
            tc.tile_pool(name="xp", bufs=4) as x_pool,
            tc.tile_pool(name="ep", bufs=2) as e_pool,
            tc.tile_pool(name="sp", bufs=3) as s_pool,
            tc.tile_pool(name="smallp", bufs=4) as small_pool,
            tc.tile_pool(name="psx", bufs=7, space="PSUM") as psx_pool,
            tc.tile_pool(name="pse", bufs=1, space="PSUM") as pse_pool,
        ):
            # All conv coefficient tiles for a batch are overlapping windows
            # of qrev inside ONE [128, 1920] window: W[p, c] = qrev[lb, p+c];
            # the rhs for (kj, tch) is W[:, 512*tch + 128*(NKJ-1-kj) :+ 512].
            def load_w_piece(w_sb, lb, c0, c1, eng=None):
                (eng or nc.sync).dma_start(
                    out=w_sb[:, c0:c1],
                    in_=bass.AP(tensor=qr_d, offset=lb * QLEN + c0,
                                ap=[[1, 128], [1, c1 - c0]]),
                )

            def load_w(lb):
                w_sb = conv_pool.tile([128, 1920], fp8, tag="conv",
                                      name=f"w_{lb}")
                load_w_piece(w_sb, lb, 0, 1920)
                return w_sb

            g_sb = const_pool.tile([128, NKJ, DIM_W], fp8)

            def load_g(s0, s1, eng=None):  # slot range [s0, s1): one DMA
                # tile-slice out AP so reads of other slots don't pick up a
                # false dependency on this DMA
                (eng or nc.sync).dma_start(
                    out=g_sb[:, s0:s1, :],
                    in_=bass.AP(tensor=g_d, offset=s0 * DIM_W,
                                ap=[[NKJ * DIM_W, 128], [1, (s1 - s0) * DIM_W]]),
                )

            # h loads: one packed DMA per batch (8 KiB per partition line);
            # the bf16 tile feeds the Vh matmuls directly.  Batch 0 splits
            # per-kd so the early-kd Vh matmuls can start (keeping the PE
            # busy) while the later slices are still in flight.
            def load_ht(lb, split_dma=False):
                htb_sb = htb_pool.tile([128, NKD * T], bf16, tag="htb",
                                       name=f"htb_{lb}")
                if split_dma:
                    for kd in range(NKD):
                        nc.sync.dma_start(
                            out=htb_sb[:, kd * T:(kd + 1) * T],
                            in_=bass.AP(tensor=ht_d,
                                        offset=lb * 128 * NKD * T + kd * T,
                                        ap=[[NKD * T, 128], [1, T]]),
                        )
                else:
                    nc.sync.dma_start(
                        out=htb_sb[:],
                        in_=bass.AP(tensor=ht_d, offset=lb * 128 * NKD * T,
                                    ap=[[NKD * T, 128], [1, NKD * T]]),
                    )
                return htb_sb

            vw_sb = const_pool.tile([128, NKD, DIM_W], bf16)
            bias_sb = const_pool.tile([128, B_LOC * 4], f32)
            wwr_sb = const_pool.tile([128, 5], f32r)

            # HAM warmup: the PE idles ~4us waiting for the first loads, and
            # whatever runs in the first ~3.4us of PE activity runs at the
            # cold 1.2GHz clock.  Spend that window on dummy matmuls over a
            # zeroed scratch tile so the real matmuls start at 2.4GHz; the
            # dummies and the real stream must stay gap-free (a fully-idle
            # 3.4us HAM window re-throttles).
            warm_in = const_pool.tile([128, 640], bf16)
            nc.vector.memset(warm_in[:], 0.0)

            # Startup DMA issue splits across BOTH HWDGE sequencers so the
            # two first-needed transfers (g pairs 3,2 and the w0 window)
            # land together ~6us in: Sync takes g + ht0, Scalar takes w0 +
            # vw + constants.  Per-DMA issue-to-complete is ~5.3us here, so
            # issue order IS the completion order.
            # tch0's conv windows span w cols [0, 1408): pair p at tch reads
            # cols [512*tch + 128*(6-2p), +640).
            # CAUTION: this exact issue order matters beyond overlap — runs
            # that front-load the 1MB ht0 stream into the first issue slots
            # (before g-lo) latch the WHOLE run into the P0 power state
            # (PE at ~2.0GHz instead of 2.4 -> +16us).  Measured repeatedly;
            # keep ht0 after g-lo even though its late slices cost ~2.5us.
            # (g-hi via GpSimd/SWDGE was tried: the SWDGE transfer lands
            # LATER than HWDGE's, net +1.6us.  All loads stay HWDGE.)
            load_g(4, NKJ)                 # pairs 3,2 (first conv matmuls)
            w0_sb = conv_pool.tile([128, 1920], fp8, tag="conv", name="w_0")
            load_w_piece(w0_sb, 0, 0, 1408, eng=nc.scalar)  # tch0 windows
            load_g(0, 4)                        # pairs 1,0
            ht0_sb = load_ht(0, split_dma=True)
            load_w_piece(w0_sb, 0, 1408, 1920)  # rest (tch1 windows)

            warm_ps = psx_pool.tile([128, 512], f32, tag="psx", name="warm_ps")
            for _ in range(N_WARM):
                nc.tensor.matmul(warm_ps[:], warm_in[:, 0:128],
                                 warm_in[:, 128:640], start=True, stop=True)
            # Dummy activations so the Tanh/Exp act-table loads happen during
            # the startup DMA wait instead of right before the first use.
            aw_sb = small_pool.tile([1, 2], f32, tag="aw", name="aw")
            nc.scalar.activation(aw_sb[:, 0:1], warm_in[0:1, 0:1], AFT.Tanh)
            nc.scalar.activation(aw_sb[:, 1:2], warm_in[0:1, 0:1], AFT.Exp)

            nc.scalar.dma_start(
                out=vw_sb[:],
                in_=bass.AP(tensor=vw_d, offset=0,
                            ap=[[NKD * DIM_W, 128], [1, NKD * DIM_W]]),
            )
            nc.scalar.dma_start(out=bias_sb[:], in_=bias_d.ap()[:])
            nc.scalar.dma_start(out=wwr_sb[:], in_=wwr_d.ap()[:])

            # --- per-batch emission helpers -------------------------------
            # Conv runs in fp8e4 DoubleRow: each matmul contracts a PAIR of
            # 128-K-blocks at 2 rows/cycle.  g_sb slot order is
            # host-arranged [kj1,kj0,kj3,kj2,...] so pair p's dim1 is
            # (kj_hi=2p+1, kj_lo=2p) matching the rhs windows at
            # (c0, c0+128).  Pairs p>=2 first: at tch==0 their t=0
            # coefficients are naturally zero, so the start=True matmul
            # covers the full 512 columns; pairs p<2 (kj<4) carry junk t=0
            # coefficients (the t=0 conv output row is zero by construction
            # -- even-T padding in the reference), so they skip column 0
            # (N=511, odd rhs offset) and just accumulate.
            # Pair-major emission (all wt groups' pair p before pair p-1):
            # if a load for a later pair is still in flight, every ready
            # matmul ahead of it still issues (the PE queue is in-order, so
            # group-major order would head-of-line-block on the g/w DMAs
            # during batch 0).
            def emit_conv_pair(w_sb, pss, tch, p, wts, start):
                c0 = 512 * tch + 128 * (NKJ - 2 - 2 * p)
                skip = 1 if (tch == 0 and p < 2) else 0
                rhs = bass.AP(
                    tensor=w_sb.tensor,
                    offset=w_sb.offset + c0 + skip,
                    ap=[list(w_sb.ap[0]), [128, 2], [1, 512 - skip]],
                )
                for wt in wts:
                    nc.tensor.matmul(
                        pss[wt][:, skip:],
                        g_sb[:, 2 * p:2 * p + 2, wt * 128:(wt + 1) * 128],
                        rhs, start=start, stop=False, perf_mode=DR,
                    )

            def emit_conv(w_sb, pss, tch, pairs=(3, 2, 1, 0)):
                for p in pairs:
                    emit_conv_pair(w_sb, pss, tch, p, range(NWT), p == 3)

            def emit_vh(htb_sb, ps, tch, wt):
                for kd in range(NKD):
                    c = kd * T + tch * 512
                    nc.tensor.matmul(
                        ps[:],
                        vw_sb[:, kd, wt * 128:(wt + 1) * 128],
                        htb_sb[:, c:c + 512],
                        start=False, stop=(kd == NKD - 1),
                    )

            def emit_act(lb, ps, x_sb, wt):
                nc.scalar.activation(
                    x_sb[:, wt, :], ps[:], AFT.Tanh,
                    bias=bias_sb[:, lb * 4 + wt: lb * 4 + wt + 1],
                    scale=1.0 / SC,
                )

            def emit_e_offload(lb, tch, pe_t, x_sb):
                # Reduce over wt on the Vector engine (s = sum_wt
                # x_wt * ww_wt), then one K=128 ones-matmul does the
                # partition sum: frees ~1.4us/batch of PE time.
                # (gpsimd.partition_all_reduce was measured at 3.5us per
                # call -- 16x the ones-matmul -- so the PE hop stays.  A
                # bf16 ones-matmul variant measured net-slower: the 400ns
                # block-boundary gaps are dependency waits, not dtype-mode
                # switches, and the bf16 s hop added two ~309ns waits.)
                s_sb = s_pool.tile([128, 512], f32r, tag="s",
                                   name=f"s_{lb}_{tch}")
                nc.vector.tensor_scalar_mul(
                    s_sb[:], x_sb[:, 0, :].bitcast(f32),
                    wwr_sb[:, 0:1].bitcast(f32))
                for wt in range(1, NWT):
                    nc.vector.scalar_tensor_tensor(
                        s_sb[:], x_sb[:, wt, :].bitcast(f32),
                        wwr_sb[:, wt:wt + 1].bitcast(f32),
                        s_sb[:].bitcast(f32),
                        op0=mybir.AluOpType.mult,
                        op1=mybir.AluOpType.add)
                nc.tensor.matmul(pe_t[:], wwr_sb[:, 4:5], s_sb[:],
                                 start=True, stop=True)
                return pe_t[:]

            def emit_e_pe(pe_t, x_sb):
                for wt in range(NWT):
                    nc.tensor.matmul(
                        pe_t[:], wwr_sb[:, wt:wt + 1], x_sb[:, wt, :],
                        start=(wt == 0), stop=(wt == NWT - 1),
                    )
                return pe_t[:]

            # beta is small (|e| <~ 20): exp(beta*e) cannot overflow, so no
            # max-subtraction pass; each t-half is exponentiated and shipped
            # right after its e matmul.  Host sums + divides.
            assert abs(beta) <= 4.0, "large-beta path removed (host softmax)"

            def emit_exp_out(lb, tch, e_src, p_sb, raw=False):
                if raw:
                    # tail-exposed slice: ship RAW e via a ~200ns DVE copy
                    # (PSUM->SBUF) instead of the 678ns Scalar exp; the host
                    # applies exp to this one slice (see _RAW_SLICE).
                    nc.vector.tensor_copy(
                        out=p_sb[:, tch * 512:(tch + 1) * 512], in_=e_src)
                else:
                    nc.scalar.activation(
                        p_sb[:, tch * 512:(tch + 1) * 512],
                        e_src, AFT.Exp, scale=float(beta),
                    )
                nc.sync.dma_start(
                    out=out_d.ap()[lb:lb + 1, tch * 512:(tch + 1) * 512],
                    in_=p_sb[:, tch * 512:(tch + 1) * 512])

            # Each block's EXP is emitted one block LATE: the Scalar engine
            # is FIFO, and an exp placed between two blocks' tanh groups
            # stalls the later tanhs on the e-reduction chain (DVE s ops ->
            # ones-matmul) it waits for; that delays the PSUM-bank frees the
            # next conv group needs.  Deferring the exp one block gives its
            # inputs a full block of slack.  The last batch flushes inline
            # (shortest exposed tail).
            pending_exp = None
            fill1 = None  # batch-0 tch1 PSUM tiles pre-started during tch0
            for lb in range(B_LOC):
                htb_sb = ht0_sb if lb == 0 else load_ht(lb)
                w_sb = w0_sb if lb == 0 else load_w(lb)
                p_sb = e_pool.tile([1, T], f32, tag="p", name=f"p_{lb}")
                offload = lb < B_LOC - 1
                for tch in range(NTC):
                    x_sb = x_pool.tile([128, NWT, 512], f32r, tag="x",
                                       name=f"x_{lb}_{tch}")
                    if fill1 is not None and tch == 1:
                        pss = fill1
                        fill1 = None
                        # finish what the tch0-time fill couldn't start:
                        # wt3 (its PSUM bank frees with tanh(tch0,wt0)) and
                        # pairs 1,0 (which need the w0B window).
                        emit_conv_pair(w_sb, pss, 1, 3, [3], True)
                        emit_conv_pair(w_sb, pss, 1, 2, [3], False)
                        emit_conv(w_sb, pss, 1, pairs=(1, 0))
                    else:
                        # allocate in reverse wt order: the pool recycles
                        # banks in allocation order, so the freshest (still
                        # tanh-pending) bank lands on wt3, whose conv-start
                        # is emitted last (+3 matmul slots of extra slack)
                        pss = [None] * NWT
                        for wt in reversed(range(NWT)):
                            pss[wt] = psx_pool.tile(
                                [128, 512], f32, tag="psx",
                                name=f"ps_{lb}_{tch}_{wt}")
                        # NOTE: interleaving pair 0 with the Vh groups (to
                        # advance each group's stop) measured net-SLOWER
                        # even at full clock (77.1us, stream excess 8.1us
                        # vs 3.6us) — the per-group pair-0 emission creates
                        # more wait exposure than it removes.  Keep the
                        # plain pair-major order.
                        emit_conv(w_sb, pss, tch)
                    if lb == 0 and tch == 0:
                        # Fill the ht0-landing wait with tch1 conv work that
                        # needs only the already-loaded g + w0A data: pairs
                        # 3,2 for wt 0..2 (wt3's bank isn't free yet).
                        fill1 = []
                        for wt in range(NWT):
                            ps = psx_pool.tile([128, 512], f32, tag="psx",
                                               name=f"ps_0_1_{wt}")
                            fill1.append(ps)
                        emit_conv_pair(w_sb, fill1, 1, 3, [0, 1, 2], True)
                        emit_conv_pair(w_sb, fill1, 1, 2, [0, 1, 2], False)
                    for wt in range(NWT):
                        emit_vh(htb_sb, pss[wt], tch, wt)
                        emit_act(lb, pss[wt], x_sb, wt)
                    if pending_exp is not None:
                        emit_exp_out(*pending_exp)
                        pending_exp = None
                    # the last batch's tch1 e stays on the PE (the DVE hop
                    # would lengthen the exposed tail); its tch0 e is not
                    # tail-exposed, so it offloads like the others
                    pe_t = pse_pool.tile([1, 512], f32, tag="pse",
                                         name=f"pe_{lb}_{tch}")
                    if offload or tch == 0:
                        e_src = emit_e_offload(lb, tch, pe_t, x_sb)
                    else:
                        e_src = emit_e_pe(pe_t, x_sb)
                    if lb == B_LOC - 1 and tch == 1:
                        # raw-e shipping (DVE copy + host exp) for this
                        # slice was tried and measured neutral-to-worse
                        # (76.3-76.7 vs 74.9-75.9); the tail is quantized
                        # by the end-barrier + epilogue, so keep the exp.
                        # The last batch's tch0 exp defers like the others:
                        # emitted inline it sits in the Scalar FIFO ahead
                        # of tch1's tanhs, stalling them on its e-chain.
                        emit_exp_out(lb, tch, e_src, p_sb)
                    else:
                        pending_exp = (lb, tch, e_src, p_sb)

    nc.compile()
    return nc


def _get_program(beta: float):
    if beta not in _program_cache:
        _program_cache[beta] = _build_program(beta)
    return _program_cache[beta]


def _prepare_in_maps(F, a_prev, s_prev, h, Ww, Vw, Vb, Uw, ww):
    """Host-side sharding + layout prep. Cheap (one small matmul + copies)."""
    import ml_dtypes
    e4 = ml_dtypes.float8_e4m3
    bf16 = ml_dtypes.bfloat16
    G = (F.astype(np.float64) @ Uw.astype(np.float64)).astype(np.float32)
    # Reverse each 128-row block of G so conv lhsT/rhs partition orders match,
    # then swap each even/odd kj block so DoubleRow pair p's lhsT dim1 order
    # is (kj_hi=2p+1, kj_lo=2p), matching rhs windows at (c0, c0+128).
    G_br = G.reshape(T // 128, 128, DIM_W)[:, ::-1, :]
    G_br = G_br[[1, 0, 3, 2, 5, 4, 7, 6]]
    # partition-major packing: g_pk[p, slot, :] = G_br[slot*128 + p, :]
    g_pk = np.ascontiguousarray(
        G_br.astype(e4).transpose(1, 0, 2).reshape(128, T // 128 * DIM_W))
    Ws = (s_prev.astype(np.float64) @ Ww.astype(np.float64)).astype(np.float32)
    Ws = Ws + Vb[None, :]                                   # [B, DIM_W]
    # Vh accumulates into the same PSUM as the SC-scaled fp8 conv, so Vw is
    # pre-scaled by SC (exact, power of 2); the tanh activation undoes it.
    # partition-major packing: vw_pk[p, kd, :] = Vw[kd*128 + p, :] * SC
    vw_pk = np.ascontiguousarray(
        (Vw * np.float32(SC)).astype(bf16).reshape(4, 128, DIM_W)
        .transpose(1, 0, 2).reshape(128, 4 * DIM_W))

    in_maps = []
    for core in range(N_CORES):
        b0 = core * B_LOC
        ppad = np.zeros((B_LOC, 2 * T - 1), np.float32)
        ppad[:, T // 2 - 1: T // 2 - 1 + T] = a_prev[b0:b0 + B_LOC]
        qrev = np.zeros((B_LOC, QLEN), np.float32)
        qrev[:, : 2 * T - 1] = ppad[:, ::-1]
        qrev = (qrev * np.float32(SC)).astype(e4)
        # ht_pk[lb, p, kd, t] = h[t, b0+lb, kd*128+p]  (partition-major)
        hT = (h[:, b0:b0 + B_LOC, :].transpose(1, 2, 0).astype(bf16)
              .reshape(B_LOC, 4, 128, T).transpose(0, 2, 1, 3)
              .reshape(B_LOC, 128 * 4 * T))
        hT = np.ascontiguousarray(hT)
        bias_core = np.ascontiguousarray(
            Ws[b0:b0 + B_LOC].reshape(B_LOC, 4, 128)
            .transpose(2, 0, 1).reshape(128, B_LOC * 4))
        # [128, 5]: cols 0-3 = ww blocks, col 4 = ones (e partition-sum lhsT)
        wwr = np.ascontiguousarray(np.concatenate(
            [ww.reshape(4, 128).T, np.ones((128, 1), np.float32)], axis=1))
        in_maps.append({
            "g": g_pk, "vw": vw_pk, "qrev": qrev, "ht": hT,
            "bias": bias_core, "wwr": wwr,
        })
    return in_maps


def _finish(p_cores: list, beta: float) -> np.ndarray:
    """p_cores: per-core [B_LOC, T] unnormalized exp(beta*e) rows from the
    device. Host does the softmax sum+divide (the reference's exact tail)."""
    p = np.concatenate(p_cores, axis=0).astype(np.float64)
    return (p / p.sum(axis=1, keepdims=True)).astype(np.float32)


def kernel(**inputs: np.ndarray) -> np.ndarray:
    F = np.ascontiguousarray(np.asarray(inputs["F_matrix"], dtype=np.float32))
    a_prev = np.ascontiguousarray(np.asarray(inputs["a_prev"], dtype=np.float32))
    s_prev = np.ascontiguousarray(np.asarray(inputs["s_prev"], dtype=np.float32))
    h = np.ascontiguousarray(np.asarray(inputs["h"], dtype=np.float32))
    Ww = np.asarray(inputs["Ww"], dtype=np.float32)
    Vw = np.asarray(inputs["Vw"], dtype=np.float32)
    Vb = np.asarray(inputs["Vb"], dtype=np.float32)
    Uw = np.asarray(inputs["Uw"], dtype=np.float32)
    ww = np.asarray(inputs["ww"], dtype=np.float32)
    beta = float(np.asarray(inputs["beta"]))

    nc = _get_program(beta)
    in_maps = _prepare_in_maps(F, a_prev, s_prev, h, Ww, Vw, Vb, Uw, ww)

    from concourse.bass_utils import run_bass_kernel_spmd

    res = run_bass_kernel_spmd(nc, in_maps, core_ids=list(range(N_CORES)))
    return _finish([res.results[i]["out"] for i in range(N_CORES)], beta)
